# revision 20
# baseline (speedup 1.0000x reference)
"""Trainium2 Bass kernel for nn_AttentionBlock (B=32, C=1024, H=W=32, nh=1).

Reference computation (per batch b, with S = H*W = 1024):
    qkv = w_qkv @ x_b            # [3C, S], 1x1 conv == channel matmul
    q, k, v = split(qkv)
    logits[t,s] = (q[:,t] . k[:,s]) / sqrt(C)
    attn = softmax_s(logits)
    h[t,s] = attn[t,s] * sum_c v[c,s]
    out = w_proj @ h + b_proj + x_b

Algebraic simplifications (weight/host-side precompute):
  * logits = x^T (M x) with M = Wq^T Wk  -> q/k never materialized.
  * vs[s] = sum_c v[c,s] = (sum_c Wv) . x[:,s] — cheap, computed on host
    (like M itself) and shipped as an fp16 [P,S] broadcast plane.
  * softmax row-normalization is folded into the projection weights:
    out = ((Wp^T * rcp) @ e) .* vs + (x + b) with e = exp(scale*l - ln4).
  * residual+bias (x + b_proj) precomputed on host in fp16.

Precision (fp8 e4m3 DoubleRow = 2x PE throughput, measured on HW):
  * Stage A (y16 = 16*M^T x): fp8 DR, fp32 psum; y16 requantized to fp8.
  * Stage B (l16 = x8^T y8): fp8 DR.
  * exp activation writes e8 (fp8) directly, with a -ln4 input bias so the
    max value stays ~4x under e4m3's 240 (beyond which TRN gives Inf);
    the bias self-cancels through the row-sum normalization (accum_out).
  * Stage C (proj = wps8 @ e8): fp8 DR; wps8 = (host 2^16*Wp^T, fp16) *
    rcp quantized on DVE per row-block; 2^-16 folded into the host vs.
  * Output fp16, upcast to fp32 on host. Measured rel err: 1.10e-2.

Engine placement (all measured on HW): y8 copies + psum downcasts on ACT
(DVE's in-order queue would stall the PE behind the previous stage's
work); wps8 scaling + the vs-multiply/residual-add on DVE; GpSimd is
~14x slower than DVE for elementwise and is not used.

Sharding: data-parallel over batch, 4 batches per core on 8 cores.
"""

import os
import sys

import numpy as np

for _p in ("/opt/trn_rl_repo", "/opt/pypackages"):
    if _p not in sys.path:
        sys.path.insert(0, _p)

import ml_dtypes

import concourse.bass as bass
import concourse.tile as tile
from concourse import bacc, mybir
from concourse.bass_utils import run_bass_kernel_spmd
from concourse.tile_rust import add_dep_helper

B, C, HH, WW = 32, 1024, 32, 32
S = HH * WW          # 1024 spatial positions
P = 128              # partitions
KC = C // P          # 8 chunks along channel dim
TC = S // P          # 8 chunks along spatial (t) dim
QC = C // 256        # 4 DoubleRow chunks along contraction dim
NN = 512             # matmul moving free dim
NCH = S // NN        # 2 free-dim halves
N_CORES = 8
BPC = B // N_CORES   # batches per core
A_SCALE = 16.0       # host pre-scale of M for fp8 range
SCALE = 1.0 / (np.sqrt(float(C)) * A_SCALE)  # folded into the exp
LN4 = float(np.log(4.0))
WPS = float(2 ** 16)  # host pre-scale of Wp^T (rcp folding keeps fp8 normal)

f32 = mybir.dt.float32
f16 = mybir.dt.float16
fp8 = mybir.dt.float8e4

N_WARMUP = int(os.environ.get("KERNEL_WARMUP", "150"))
N_FILLER = int(os.environ.get("KERNEL_FILLER", "60"))


def build_nc(bpc: int = BPC):
    nc = bacc.Bacc(
        "TRN2",
        target_bir_lowering=False,
        debug=False,
        enable_asserts=False,
    )

    # x in fp8 DoubleRow layout [q, p, i, s]: channel c = q*256 + i*128 + p
    x8_d = nc.dram_tensor("x8", [bpc, QC, P, 2, S], fp8, kind="ExternalInput")
    # host-precomputed f16(x + b_proj), chunk layout [k, p, s]: c = k*128 + p
    xpb_d = nc.dram_tensor("xpb", [bpc, KC, P, S], f16, kind="ExternalInput")
    # A16 in SBUF layout [p][mc][q][i][m]: lhsT for stage A (fp8, 16*Wk^T Wq)
    a16_d = nc.dram_tensor("a16", [P, KC, QC, 2, P], fp8, kind="ExternalInput")
    # w_proj^T * WPS stripes: [tt][p][o]
    wpt_d = nc.dram_tensor("wpt", [TC, P, C], f16, kind="ExternalInput")
    # host-precomputed vs/WPS, replicated across partitions: [p][s]
    vsf_d = nc.dram_tensor("vsf", [bpc, P, S], f16, kind="ExternalInput")
    out_d = nc.dram_tensor("out", [bpc, C, S], f16, kind="ExternalOutput")

    with tile.TileContext(nc) as tc:
        with (
            tc.tile_pool(name="weights", bufs=1) as wpool,
            tc.tile_pool(name="x8", bufs=2) as x8pool,
            tc.tile_pool(name="xpb", bufs=2) as xppool,
            tc.tile_pool(name="y", bufs=1) as ypool,
            tc.tile_pool(name="e", bufs=1) as epool,
            tc.tile_pool(name="wpts", bufs=1) as wptspool,
            tc.tile_pool(name="vsb", bufs=2) as vpool,
            tc.tile_pool(name="osb", bufs=4) as opool,
            tc.tile_pool(name="small", bufs=40) as spool,
            tc.tile_pool(name="psA", bufs=3, space="PSUM") as psA,
            tc.tile_pool(name="psB", bufs=3, space="PSUM") as psB,
            tc.tile_pool(name="psC", bufs=2, space="PSUM") as psC,
        ):
            # warm the PE clock (HAM) with throwaway matmuls on a memset
            # tile — no DMA dependency, so they start immediately
            wz = wpool.tile([P, P], f16, tag="wz")
            nc.vector.memset(wz[:], 0.25)
            ln4t = wpool.tile([P, 1], f32, tag="ln4")
            nc.vector.memset(ln4t[:], -LN4)
            wu = psA.tile([P, NN], f32, tag="psA")
            for _ in range(N_WARMUP):
                nc.tensor.matmul(
                    wu[:, 0:64], wz[:], wz[:, 0:64],
                    start=True, stop=True,
                )
            a16_sb = wpool.tile([P, KC, QC, 2, P], fp8, tag="a16")
            wpt_sb = wpool.tile([P, TC, C], f16, tag="wpt")
            x8_next = xpb_next = vsb_next = None

            for b in range(bpc):
                if b == 0:
                    x8t = x8pool.tile([P, QC, 2, S], fp8, tag="x8")
                    xpb = xppool.tile([P, KC, S], f16, tag="xpb")
                    vsb = vpool.tile([P, S], f16, tag="vsb")
                    # Critical startup set: a16 on the sync queue, x8 on the
                    # ACT hwdge queue — both issue in parallel; x8 split in
                    # n-halves so the first psum group waits on half the
                    # bytes. Non-critical loads are emitted after the first
                    # matmul so they don't dilute ring bandwidth.
                    nc.sync.dma_start(a16_sb[:, 0:1], a16_d[:, 0:1])
                    for q in range(QC):
                        nc.scalar.dma_start(
                            x8t[:, q, :, 0:NN], x8_d[b, q, :, :, 0:NN]
                        )
                    nc.sync.dma_start(a16_sb[:, 1:KC], a16_d[:, 1:KC])
                    for q in range(QC):
                        nc.sync.dma_start(
                            x8t[:, q, :, NN:S], x8_d[b, q, :, :, NN:S]
                        )
                else:
                    # tiles + DMAs were issued during the previous batch
                    # (ahead of its output DMAs in the sync queue)
                    x8t, xpb, vsb = x8_next, xpb_next, vsb_next

                # ---- stage A: y16 = (16 M^T) x via fp8 DoubleRow ----
                y8 = ypool.tile([P, QC, 2, S], fp8, tag="y8")
                for n in range(NCH):
                    for mc in range(KC):
                        ps = psA.tile([P, NN], f32, tag="psA")
                        for q in range(QC):
                            mm = nc.tensor.matmul(
                                ps[:],
                                a16_sb[:, mc, q, :, :],
                                x8t[:, q, :, n * NN : (n + 1) * NN],
                                start=(q == 0),
                                stop=(q == QC - 1),
                                perf_mode=mybir.MatmulPerfMode.DoubleRow,
                            )
                            if b == 0 and mc == 4 and q == 0 and n == 1:
                                # by n=1 mc=4 every critical startup byte
                                # has landed and been consumed for a while
                                h1_mm = mm.ins
                        # y8 copies on ACT: DVE's in-order queue is still
                        # draining the previous batch's stage-C osb chain,
                        # which would stall these (and the PE behind them)
                        nc.scalar.activation(
                            y8[:, mc // 2, mc % 2, n * NN : (n + 1) * NN],
                            ps[:],
                            mybir.ActivationFunctionType.Copy,
                        )
                        if b == 0 and n == 0 and mc == 0:
                            # keep the PE busy (HAM warm) while the remaining
                            # A16 stripes stream in
                            wuf = psA.tile([P, NN], f32, tag="psA")
                            for _ in range(N_FILLER):
                                nc.tensor.matmul(
                                    wuf[:, 0:64], wz[:], wz[:, 0:64],
                                    start=True, stop=True,
                                )
                        if b == 0 and n == 1 and mc == 4:
                            # non-critical input loads start only once ALL
                            # critical x8/a16 bytes have landed (n=1 matmul
                            # consumes the x8 h1 half), so they don't steal
                            # ring bandwidth from the critical stream
                            noncrit = [
                                nc.scalar.dma_start(
                                    xpb[:],
                                    xpb_d.rearrange("b k p s -> b p k s")[b],
                                ),
                                nc.scalar.dma_start(
                                    wpt_sb[:],
                                    wpt_d.rearrange("t p o -> p t o"),
                                ),
                                nc.scalar.dma_start(vsb[:], vsf_d[b]),
                            ]
                            for inst in noncrit:
                                add_dep_helper(
                                    inst.ins, h1_mm, sync=True,
                                    reason="startup: after critical DMAs",
                                )

                # ---- stage B: l16 = x8^T y8 (fp8 DR); exp -> e8, row sums;
                # wps8 = wpt * rcp (DVE; ACT is busy with exp in this
                # window) ----
                e8 = epool.tile([P, QC, 2, S], fp8, tag="e8")
                wps8 = wptspool.tile([P, QC, 2, C], fp8, tag="wps8")
                for tt in range(TC):
                    rsh = []
                    for n in range(NCH):
                        psl = psB.tile([P, NN], f32, tag="psB")
                        for q in range(QC):
                            nc.tensor.matmul(
                                psl[:],
                                x8t[:, q, :, tt * P : (tt + 1) * P],
                                y8[:, q, :, n * NN : (n + 1) * NN],
                                start=(q == 0),
                                stop=(q == QC - 1),
                                perf_mode=mybir.MatmulPerfMode.DoubleRow,
                            )
                        rs = spool.tile([P, 1], f32, tag="rs")
                        nc.scalar.activation(
                            e8[:, tt // 2, tt % 2, n * NN : (n + 1) * NN],
                            psl[:],
                            mybir.ActivationFunctionType.Exp,
                            scale=float(SCALE), bias=ln4t[:], accum_out=rs[:],
                        )
                        rsh.append(rs)
                    rst = spool.tile([P, 1], f32, tag="rst")
                    nc.vector.tensor_tensor(
                        rst[:], rsh[0][:], rsh[1][:], mybir.AluOpType.add
                    )
                    rcp = spool.tile([P, 1], f32, tag="rcp")
                    nc.vector.reciprocal(rcp[:], rst[:])
                    nc.vector.tensor_scalar(
                        wps8[:, tt // 2, tt % 2, :], wpt_sb[:, tt, :],
                        rcp[:], None,
                        mybir.AluOpType.mult,
                    )

                # ---- prefetch next batch's inputs (ahead of this batch's
                # output DMAs in the sync queue) ----
                if b + 1 < bpc:
                    x8_next = x8pool.tile([P, QC, 2, S], fp8, tag="x8")
                    xpb_next = xppool.tile([P, KC, S], f16, tag="xpb")
                    vsb_next = vpool.tile([P, S], f16, tag="vsb")
                    nc.sync.dma_start(
                        x8_next[:], x8_d.rearrange("b q p i s -> b p q i s")[b + 1]
                    )
                    nc.sync.dma_start(
                        xpb_next[:], xpb_d.rearrange("b k p s -> b p k s")[b + 1]
                    )
                    nc.sync.dma_start(vsb_next[:], vsf_d[b + 1])

                # ---- stage C: out = (wps8 @ e8) * vs + (x + b) ----
                cpools = (
                    [(psC, "psC"), (psA, "psA"), (psB, "psB")]
                    if b == bpc - 1
                    else [(psC, "psC")]
                )
                for oc in range(KC):
                    for n in range(NCH):
                        cp, ctag = cpools[(oc * NCH + n) % len(cpools)]
                        pso = cp.tile([P, NN], f32, tag=ctag)
                        for q in range(QC):
                            nc.tensor.matmul(
                                pso[:],
                                wps8[:, q, :, oc * P : (oc + 1) * P],
                                e8[:, q, :, n * NN : (n + 1) * NN],
                                start=(q == 0),
                                stop=(q == QC - 1),
                                perf_mode=mybir.MatmulPerfMode.DoubleRow,
                            )
                        # psum downcast so DVE's multiply runs on
                        # all-16-bit operands; the first two go on DVE
                        # (idle at C start) since ACT is still finishing
                        # the last exps of stage B
                        os16 = opool.tile([P, NN], f16, tag="os16")
                        if oc * NCH + n < 2:
                            nc.vector.tensor_copy(out=os16[:], in_=pso[:])
                        else:
                            nc.scalar.activation(
                                os16[:], pso[:],
                                mybir.ActivationFunctionType.Copy,
                            )
                        osb = opool.tile([P, NN], f16, tag="osb")
                        nc.vector.tensor_tensor(
                            osb[:], os16[:], vsb[:, n * NN : (n + 1) * NN],
                            mybir.AluOpType.mult,
                        )
                        nc.vector.tensor_tensor(
                            osb[:], osb[:], xpb[:, oc, n * NN : (n + 1) * NN],
                            mybir.AluOpType.add,
                        )
                        nc.sync.dma_start(
                            out_d[b, oc * P : (oc + 1) * P, n * NN : (n + 1) * NN],
                            osb[:],
                        )
    nc.compile()
    return nc


def _host_prep(w_qkv, w_proj, b_proj):
    wq = w_qkv[0:C].astype(np.float64)
    wk = w_qkv[C : 2 * C].astype(np.float64)
    wv = w_qkv[2 * C : 3 * C]
    # lhsT for y-matmul: a16[d, c] = 16*M[c, d], M = Wq^T Wk => a16 = 16*Wk^T Wq
    a16 = np.clip(A_SCALE * (wk.T @ wq), -240.0, 240.0).astype(
        ml_dtypes.float8_e4m3
    )
    # SBUF layout [p][mc][q][i][m]: contraction d = q*256 + i*128 + p,
    # output col index c = mc*128 + m
    a16_s = np.ascontiguousarray(
        a16.reshape(QC, 2, P, KC, P).transpose(2, 3, 0, 1, 4)
    )
    wvs = wv.sum(axis=0, dtype=np.float64).astype(np.float32)
    # wpt[tt][p][o] = WPS * w_proj[o, t = tt*128 + p]
    wpt_s = np.ascontiguousarray(
        (w_proj.T * WPS).reshape(TC, P, C).astype(np.float16)
    )
    return a16_s, wpt_s, wvs


_NC_CACHE = {}


def _get_nc(bpc=BPC):
    if bpc not in _NC_CACHE:
        _NC_CACHE[bpc] = build_nc(bpc)
    return _NC_CACHE[bpc]


def kernel(x, w_qkv, w_proj, b_proj, _trace=False):
    x = np.asarray(x, dtype=np.float32)
    a16, wpt, wvs = _host_prep(
        np.asarray(w_qkv, np.float32),
        np.asarray(w_proj, np.float32),
        np.asarray(b_proj, np.float32),
    )
    bp = np.asarray(b_proj, np.float32)
    xr_full = x.reshape(B, C, S)
    # fp8 DR layout [b, q, p, i, s]: c = q*256 + i*128 + p
    x8_full = (
        np.clip(xr_full, -240.0, 240.0)
        .astype(ml_dtypes.float8_e4m3)
        .reshape(B, QC, 2, P, S)
        .transpose(0, 1, 3, 2, 4)
    )
    # residual + bias, f16
    xpb_full = (xr_full + bp[None, :, None]).astype(np.float16).reshape(
        B, KC, P, S
    )
    # vs/WPS as an f16 plane replicated across partitions
    vs_full = (np.einsum("c,bcs->bs", wvs, xr_full) / WPS).astype(np.float16)
    vsf_full = np.broadcast_to(vs_full[:, None, :], (B, P, S))
    in_maps = []
    for c in range(N_CORES):
        sl = slice(c * BPC, (c + 1) * BPC)
        in_maps.append(
            {
                "x8": np.ascontiguousarray(x8_full[sl]),
                "xpb": np.ascontiguousarray(xpb_full[sl]),
                "a16": a16,
                "wpt": wpt,
                "vsf": np.ascontiguousarray(vsf_full[sl]),
            }
        )
    nc = _get_nc(BPC)
    res = run_bass_kernel_spmd(
        nc, in_maps, core_ids=list(range(N_CORES)), trace=_trace
    )
    out = np.concatenate([r["out"] for r in res.results], axis=0)
    out = out.astype(np.float32).reshape(B, C, HH, WW)
    if _trace:
        kernel.last_results = res
    return out


# revision 21
# speedup vs baseline: 1.0083x; 1.0083x over previous
"""Trainium2 Bass kernel for nn_AttentionBlock (B=32, C=1024, H=W=32, nh=1).

Reference computation (per batch b, with S = H*W = 1024):
    qkv = w_qkv @ x_b            # [3C, S], 1x1 conv == channel matmul
    q, k, v = split(qkv)
    logits[t,s] = (q[:,t] . k[:,s]) / sqrt(C)
    attn = softmax_s(logits)
    h[t,s] = attn[t,s] * sum_c v[c,s]
    out = w_proj @ h + b_proj + x_b

Algebraic simplifications (weight/host-side precompute):
  * logits = x^T (M x) with M = Wq^T Wk  -> q/k never materialized.
  * vs[s] = sum_c v[c,s] = (sum_c Wv) . x[:,s] — cheap, computed on host
    (like M itself) and shipped as an fp16 [P,S] broadcast plane.
  * softmax row-normalization is folded into the projection weights:
    out = ((Wp^T * rcp) @ e) .* vs + (x + b) with e = exp(scale*l - ln4).
  * residual+bias (x + b_proj) precomputed on host in fp16.

Precision (fp8 e4m3 DoubleRow = 2x PE throughput, measured on HW):
  * Stage A (y16 = 16*M^T x): fp8 DR, fp32 psum; y16 requantized to fp8.
  * Stage B (l16 = x8^T y8): fp8 DR.
  * exp activation writes e8 (fp8) directly, with a -ln4 input bias so the
    max value stays ~4x under e4m3's 240 (beyond which TRN gives Inf);
    the bias self-cancels through the row-sum normalization (accum_out).
  * Stage C (proj = wps8 @ e8): fp8 DR; wps8 = (host 2^16*Wp^T, fp16) *
    rcp quantized on DVE per row-block; 2^-16 folded into the host vs.
  * Output fp16, upcast to fp32 on host. Measured rel err: 1.10e-2.

Engine placement (all measured on HW): y8 copies + psum downcasts on ACT
(DVE's in-order queue would stall the PE behind the previous stage's
work); wps8 scaling + the vs-multiply/residual-add on DVE; GpSimd is
~14x slower than DVE for elementwise and is not used.

Sharding: data-parallel over batch, 4 batches per core on 8 cores.
"""

import os
import sys

import numpy as np

for _p in ("/opt/trn_rl_repo", "/opt/pypackages"):
    if _p not in sys.path:
        sys.path.insert(0, _p)

import ml_dtypes

import concourse.bass as bass
import concourse.tile as tile
from concourse import bacc, mybir
from concourse.bass_utils import run_bass_kernel_spmd
from concourse.tile_rust import add_dep_helper

B, C, HH, WW = 32, 1024, 32, 32
S = HH * WW          # 1024 spatial positions
P = 128              # partitions
KC = C // P          # 8 chunks along channel dim
TC = S // P          # 8 chunks along spatial (t) dim
QC = C // 256        # 4 DoubleRow chunks along contraction dim
NN = 512             # matmul moving free dim
NCH = S // NN        # 2 free-dim halves
N_CORES = 8
BPC = B // N_CORES   # batches per core
A_SCALE = 16.0       # host pre-scale of M for fp8 range
SCALE = 1.0 / (np.sqrt(float(C)) * A_SCALE)  # folded into the exp
LN4 = float(np.log(4.0))
WPS = float(2 ** 16)  # host pre-scale of Wp^T (rcp folding keeps fp8 normal)

f32 = mybir.dt.float32
f16 = mybir.dt.float16
fp8 = mybir.dt.float8e4

N_WARMUP = int(os.environ.get("KERNEL_WARMUP", "150"))
N_FILLER = int(os.environ.get("KERNEL_FILLER", "60"))


def build_nc(bpc: int = BPC):
    nc = bacc.Bacc(
        "TRN2",
        target_bir_lowering=False,
        debug=False,
        enable_asserts=False,
    )

    # x in fp8 DoubleRow layout [q, p, i, s]: channel c = q*256 + i*128 + p
    x8_d = nc.dram_tensor("x8", [bpc, QC, P, 2, S], fp8, kind="ExternalInput")
    # host-precomputed f16(x + b_proj), chunk layout [k, p, s]: c = k*128 + p
    xpb_d = nc.dram_tensor("xpb", [bpc, KC, P, S], f16, kind="ExternalInput")
    # A16 in SBUF layout [p][mc][q][i][m]: lhsT for stage A (fp8, 16*Wk^T Wq)
    a16_d = nc.dram_tensor("a16", [P, KC, QC, 2, P], fp8, kind="ExternalInput")
    # w_proj^T * WPS stripes: [tt][p][o]
    wpt_d = nc.dram_tensor("wpt", [TC, P, C], f16, kind="ExternalInput")
    # host-precomputed vs/WPS, replicated across partitions: [p][s]
    vsf_d = nc.dram_tensor("vsf", [bpc, P, S], f16, kind="ExternalInput")
    out_d = nc.dram_tensor("out", [bpc, C, S], f16, kind="ExternalOutput")

    with tile.TileContext(nc) as tc:
        with (
            tc.tile_pool(name="weights", bufs=1) as wpool,
            tc.tile_pool(name="x8", bufs=2) as x8pool,
            tc.tile_pool(name="xpb", bufs=2) as xppool,
            tc.tile_pool(name="y", bufs=1) as ypool,
            tc.tile_pool(name="e", bufs=1) as epool,
            tc.tile_pool(name="wpts", bufs=1) as wptspool,
            tc.tile_pool(name="vsb", bufs=2) as vpool,
            tc.tile_pool(name="osb", bufs=4) as opool,
            tc.tile_pool(name="small", bufs=40) as spool,
            tc.tile_pool(name="psA", bufs=3, space="PSUM") as psA,
            tc.tile_pool(name="psB", bufs=3, space="PSUM") as psB,
            tc.tile_pool(name="psC", bufs=2, space="PSUM") as psC,
        ):
            # warm the PE clock (HAM) with throwaway matmuls on a memset
            # tile — no DMA dependency, so they start immediately
            wz = wpool.tile([P, P], f16, tag="wz")
            nc.vector.memset(wz[:], 0.25)
            ln4t = wpool.tile([P, 1], f32, tag="ln4")
            nc.vector.memset(ln4t[:], -LN4)
            wu = psA.tile([P, NN], f32, tag="psA")
            for _ in range(N_WARMUP):
                nc.tensor.matmul(
                    wu[:, 0:64], wz[:], wz[:, 0:64],
                    start=True, stop=True,
                )
            a16_sb = wpool.tile([P, KC, QC, 2, P], fp8, tag="a16")
            wpt_sb = wpool.tile([P, TC, C], f16, tag="wpt")
            x8_next = xpb_next = vsb_next = None

            for b in range(bpc):
                if b == 0:
                    x8t = x8pool.tile([P, QC, 2, S], fp8, tag="x8")
                    xpb = xppool.tile([P, KC, S], f16, tag="xpb")
                    vsb = vpool.tile([P, S], f16, tag="vsb")
                    # Critical startup set: a16 on the sync queue, x8 on the
                    # ACT hwdge queue — both issue in parallel; x8 split in
                    # n-halves so the first psum group waits on half the
                    # bytes. Non-critical loads are emitted after the first
                    # matmul so they don't dilute ring bandwidth.
                    nc.sync.dma_start(a16_sb[:, 0:1], a16_d[:, 0:1])
                    for q in range(QC):
                        nc.scalar.dma_start(
                            x8t[:, q, :, 0:NN], x8_d[b, q, :, :, 0:NN]
                        )
                    nc.sync.dma_start(a16_sb[:, 1:KC], a16_d[:, 1:KC])
                    for q in range(QC):
                        nc.sync.dma_start(
                            x8t[:, q, :, NN:S], x8_d[b, q, :, :, NN:S]
                        )
                else:
                    # tiles + DMAs were issued during the previous batch
                    # (ahead of its output DMAs in the sync queue)
                    x8t, xpb, vsb = x8_next, xpb_next, vsb_next

                # ---- stage A: y16 = (16 M^T) x via fp8 DoubleRow ----
                y8 = ypool.tile([P, QC, 2, S], fp8, tag="y8")
                for n in range(NCH):
                    for mc in range(KC):
                        ps = psA.tile([P, NN], f32, tag="psA")
                        for q in range(QC):
                            mm = nc.tensor.matmul(
                                ps[:],
                                a16_sb[:, mc, q, :, :],
                                x8t[:, q, :, n * NN : (n + 1) * NN],
                                start=(q == 0),
                                stop=(q == QC - 1),
                                perf_mode=mybir.MatmulPerfMode.DoubleRow,
                            )
                            if b == 0 and mc == 4 and q == 0 and n == 1:
                                # by n=1 mc=4 every critical startup byte
                                # has landed and been consumed for a while
                                h1_mm = mm.ins
                        # y8 copies on ACT: DVE's in-order queue is still
                        # draining the previous batch's stage-C osb chain,
                        # which would stall these (and the PE behind them)
                        nc.scalar.activation(
                            y8[:, mc // 2, mc % 2, n * NN : (n + 1) * NN],
                            ps[:],
                            mybir.ActivationFunctionType.Copy,
                        )
                        if b == 0 and n == 0 and mc == 0:
                            # keep the PE busy (HAM warm) while the remaining
                            # A16 stripes stream in
                            wuf = psA.tile([P, NN], f32, tag="psA")
                            for _ in range(N_FILLER):
                                nc.tensor.matmul(
                                    wuf[:, 0:64], wz[:], wz[:, 0:64],
                                    start=True, stop=True,
                                )
                        if b == 0 and n == 1 and mc == 4:
                            # non-critical input loads start only once ALL
                            # critical x8/a16 bytes have landed (n=1 matmul
                            # consumes the x8 h1 half), so they don't steal
                            # ring bandwidth from the critical stream
                            noncrit = [
                                nc.scalar.dma_start(
                                    wpt_sb[:],
                                    wpt_d.rearrange("t p o -> p t o"),
                                ),
                                nc.scalar.dma_start(
                                    xpb[:],
                                    xpb_d.rearrange("b k p s -> b p k s")[b],
                                ),
                                nc.scalar.dma_start(vsb[:], vsf_d[b]),
                            ]
                            for inst in noncrit:
                                add_dep_helper(
                                    inst.ins, h1_mm, sync=True,
                                    reason="startup: after critical DMAs",
                                )

                # ---- stage B: l16 = x8^T y8 (fp8 DR); exp -> e8, row sums;
                # wps8 = wpt * rcp (DVE; ACT is busy with exp in this
                # window) ----
                e8 = epool.tile([P, QC, 2, S], fp8, tag="e8")
                wps8 = wptspool.tile([P, QC, 2, C], fp8, tag="wps8")
                for tt in range(TC):
                    rsh = []
                    for n in range(NCH):
                        psl = psB.tile([P, NN], f32, tag="psB")
                        for q in range(QC):
                            nc.tensor.matmul(
                                psl[:],
                                x8t[:, q, :, tt * P : (tt + 1) * P],
                                y8[:, q, :, n * NN : (n + 1) * NN],
                                start=(q == 0),
                                stop=(q == QC - 1),
                                perf_mode=mybir.MatmulPerfMode.DoubleRow,
                            )
                        rs = spool.tile([P, 1], f32, tag="rs")
                        nc.scalar.activation(
                            e8[:, tt // 2, tt % 2, n * NN : (n + 1) * NN],
                            psl[:],
                            mybir.ActivationFunctionType.Exp,
                            scale=float(SCALE), bias=ln4t[:], accum_out=rs[:],
                        )
                        rsh.append(rs)
                    rst = spool.tile([P, 1], f32, tag="rst")
                    nc.vector.tensor_tensor(
                        rst[:], rsh[0][:], rsh[1][:], mybir.AluOpType.add
                    )
                    rcp = spool.tile([P, 1], f32, tag="rcp")
                    nc.vector.reciprocal(rcp[:], rst[:])
                    nc.vector.tensor_scalar(
                        wps8[:, tt // 2, tt % 2, :], wpt_sb[:, tt, :],
                        rcp[:], None,
                        mybir.AluOpType.mult,
                    )

                # ---- prefetch next batch's inputs (ahead of this batch's
                # output DMAs in the sync queue) ----
                if b + 1 < bpc:
                    x8_next = x8pool.tile([P, QC, 2, S], fp8, tag="x8")
                    xpb_next = xppool.tile([P, KC, S], f16, tag="xpb")
                    vsb_next = vpool.tile([P, S], f16, tag="vsb")
                    nc.sync.dma_start(
                        x8_next[:], x8_d.rearrange("b q p i s -> b p q i s")[b + 1]
                    )
                    nc.sync.dma_start(
                        xpb_next[:], xpb_d.rearrange("b k p s -> b p k s")[b + 1]
                    )
                    nc.sync.dma_start(vsb_next[:], vsf_d[b + 1])

                # ---- stage C: out = (wps8 @ e8) * vs + (x + b) ----
                cpools = (
                    [(psC, "psC"), (psA, "psA"), (psB, "psB")]
                    if b == bpc - 1
                    else [(psC, "psC")]
                )
                for oc in range(KC):
                    for n in range(NCH):
                        cp, ctag = cpools[(oc * NCH + n) % len(cpools)]
                        pso = cp.tile([P, NN], f32, tag=ctag)
                        for q in range(QC):
                            nc.tensor.matmul(
                                pso[:],
                                wps8[:, q, :, oc * P : (oc + 1) * P],
                                e8[:, q, :, n * NN : (n + 1) * NN],
                                start=(q == 0),
                                stop=(q == QC - 1),
                                perf_mode=mybir.MatmulPerfMode.DoubleRow,
                            )
                        # psum downcast so DVE's multiply runs on
                        # all-16-bit operands; the first two go on DVE
                        # (idle at C start) since ACT is still finishing
                        # the last exps of stage B
                        os16 = opool.tile([P, NN], f16, tag="os16")
                        if oc * NCH + n < 2:
                            nc.vector.tensor_copy(out=os16[:], in_=pso[:])
                        else:
                            nc.scalar.activation(
                                os16[:], pso[:],
                                mybir.ActivationFunctionType.Copy,
                            )
                        osb = opool.tile([P, NN], f16, tag="osb")
                        nc.vector.tensor_tensor(
                            osb[:], os16[:], vsb[:, n * NN : (n + 1) * NN],
                            mybir.AluOpType.mult,
                        )
                        nc.vector.tensor_tensor(
                            osb[:], osb[:], xpb[:, oc, n * NN : (n + 1) * NN],
                            mybir.AluOpType.add,
                        )
                        nc.sync.dma_start(
                            out_d[b, oc * P : (oc + 1) * P, n * NN : (n + 1) * NN],
                            osb[:],
                        )
    nc.compile()
    return nc


def _host_prep(w_qkv, w_proj, b_proj):
    wq = w_qkv[0:C].astype(np.float64)
    wk = w_qkv[C : 2 * C].astype(np.float64)
    wv = w_qkv[2 * C : 3 * C]
    # lhsT for y-matmul: a16[d, c] = 16*M[c, d], M = Wq^T Wk => a16 = 16*Wk^T Wq
    a16 = np.clip(A_SCALE * (wk.T @ wq), -240.0, 240.0).astype(
        ml_dtypes.float8_e4m3
    )
    # SBUF layout [p][mc][q][i][m]: contraction d = q*256 + i*128 + p,
    # output col index c = mc*128 + m
    a16_s = np.ascontiguousarray(
        a16.reshape(QC, 2, P, KC, P).transpose(2, 3, 0, 1, 4)
    )
    wvs = wv.sum(axis=0, dtype=np.float64).astype(np.float32)
    # wpt[tt][p][o] = WPS * w_proj[o, t = tt*128 + p]
    wpt_s = np.ascontiguousarray(
        (w_proj.T * WPS).reshape(TC, P, C).astype(np.float16)
    )
    return a16_s, wpt_s, wvs


_NC_CACHE = {}


def _get_nc(bpc=BPC):
    if bpc not in _NC_CACHE:
        _NC_CACHE[bpc] = build_nc(bpc)
    return _NC_CACHE[bpc]


def kernel(x, w_qkv, w_proj, b_proj, _trace=False):
    x = np.asarray(x, dtype=np.float32)
    a16, wpt, wvs = _host_prep(
        np.asarray(w_qkv, np.float32),
        np.asarray(w_proj, np.float32),
        np.asarray(b_proj, np.float32),
    )
    bp = np.asarray(b_proj, np.float32)
    xr_full = x.reshape(B, C, S)
    # fp8 DR layout [b, q, p, i, s]: c = q*256 + i*128 + p
    x8_full = (
        np.clip(xr_full, -240.0, 240.0)
        .astype(ml_dtypes.float8_e4m3)
        .reshape(B, QC, 2, P, S)
        .transpose(0, 1, 3, 2, 4)
    )
    # residual + bias, f16
    xpb_full = (xr_full + bp[None, :, None]).astype(np.float16).reshape(
        B, KC, P, S
    )
    # vs/WPS as an f16 plane replicated across partitions
    vs_full = (np.einsum("c,bcs->bs", wvs, xr_full) / WPS).astype(np.float16)
    vsf_full = np.broadcast_to(vs_full[:, None, :], (B, P, S))
    in_maps = []
    for c in range(N_CORES):
        sl = slice(c * BPC, (c + 1) * BPC)
        in_maps.append(
            {
                "x8": np.ascontiguousarray(x8_full[sl]),
                "xpb": np.ascontiguousarray(xpb_full[sl]),
                "a16": a16,
                "wpt": wpt,
                "vsf": np.ascontiguousarray(vsf_full[sl]),
            }
        )
    nc = _get_nc(BPC)
    res = run_bass_kernel_spmd(
        nc, in_maps, core_ids=list(range(N_CORES)), trace=_trace
    )
    out = np.concatenate([r["out"] for r in res.results], axis=0)
    out = out.astype(np.float32).reshape(B, C, HH, WW)
    if _trace:
        kernel.last_results = res
    return out


# revision 22
# speedup vs baseline: 1.0172x; 1.0089x over previous
"""Trainium2 Bass kernel for nn_AttentionBlock (B=32, C=1024, H=W=32, nh=1).

Reference computation (per batch b, with S = H*W = 1024):
    qkv = w_qkv @ x_b            # [3C, S], 1x1 conv == channel matmul
    q, k, v = split(qkv)
    logits[t,s] = (q[:,t] . k[:,s]) / sqrt(C)
    attn = softmax_s(logits)
    h[t,s] = attn[t,s] * sum_c v[c,s]
    out = w_proj @ h + b_proj + x_b

Algebraic simplifications (weight/host-side precompute):
  * logits = x^T (M x) with M = Wq^T Wk  -> q/k never materialized.
  * vs[s] = sum_c v[c,s] = (sum_c Wv) . x[:,s] — cheap, computed on host
    (like M itself) and shipped as an fp16 [P,S] broadcast plane.
  * softmax row-normalization is folded into the projection weights:
    out = ((Wp^T * rcp) @ e) .* vs + (x + b) with e = exp(scale*l - ln4).
  * residual+bias (x + b_proj) precomputed on host in fp16.

Precision (fp8 e4m3 DoubleRow = 2x PE throughput, measured on HW):
  * Stage A (y16 = 16*M^T x): fp8 DR, fp32 psum; y16 requantized to fp8.
  * Stage B (l16 = x8^T y8): fp8 DR.
  * exp activation writes e8 (fp8) directly, with a -ln4 input bias so the
    max value stays ~4x under e4m3's 240 (beyond which TRN gives Inf);
    the bias self-cancels through the row-sum normalization (accum_out).
  * Stage C (proj = wps8 @ e8): fp8 DR; wps8 = (host 2^16*Wp^T, fp16) *
    rcp quantized on DVE per row-block; 2^-16 folded into the host vs.
  * Output fp16, upcast to fp32 on host. Measured rel err: 1.10e-2.

Engine placement (all measured on HW): y8 copies + psum downcasts on ACT
(DVE's in-order queue would stall the PE behind the previous stage's
work); wps8 scaling + the vs-multiply/residual-add on DVE; GpSimd is
~14x slower than DVE for elementwise and is not used.

Sharding: data-parallel over batch, 4 batches per core on 8 cores.
"""

import os
import sys

import numpy as np

for _p in ("/opt/trn_rl_repo", "/opt/pypackages"):
    if _p not in sys.path:
        sys.path.insert(0, _p)

import ml_dtypes

import concourse.bass as bass
import concourse.tile as tile
from concourse import bacc, mybir
from concourse.bass_utils import run_bass_kernel_spmd
from concourse.tile_rust import add_dep_helper

B, C, HH, WW = 32, 1024, 32, 32
S = HH * WW          # 1024 spatial positions
P = 128              # partitions
KC = C // P          # 8 chunks along channel dim
TC = S // P          # 8 chunks along spatial (t) dim
QC = C // 256        # 4 DoubleRow chunks along contraction dim
NN = 512             # matmul moving free dim
NCH = S // NN        # 2 free-dim halves
N_CORES = 8
BPC = B // N_CORES   # batches per core
A_SCALE = 16.0       # host pre-scale of M for fp8 range
SCALE = 1.0 / (np.sqrt(float(C)) * A_SCALE)  # folded into the exp
LN4 = float(np.log(4.0))
WPS = float(2 ** 16)  # host pre-scale of Wp^T (rcp folding keeps fp8 normal)

f32 = mybir.dt.float32
f16 = mybir.dt.float16
fp8 = mybir.dt.float8e4

N_WARMUP = int(os.environ.get("KERNEL_WARMUP", "100"))
N_FILLER = int(os.environ.get("KERNEL_FILLER", "60"))


def build_nc(bpc: int = BPC):
    nc = bacc.Bacc(
        "TRN2",
        target_bir_lowering=False,
        debug=False,
        enable_asserts=False,
    )

    # x in fp8 DoubleRow layout [q, p, i, s]: channel c = q*256 + i*128 + p
    x8_d = nc.dram_tensor("x8", [bpc, QC, P, 2, S], fp8, kind="ExternalInput")
    # host-precomputed f16(x + b_proj), chunk layout [k, p, s]: c = k*128 + p
    xpb_d = nc.dram_tensor("xpb", [bpc, KC, P, S], f16, kind="ExternalInput")
    # A16 in SBUF layout [p][mc][q][i][m]: lhsT for stage A (fp8, 16*Wk^T Wq)
    a16_d = nc.dram_tensor("a16", [P, KC, QC, 2, P], fp8, kind="ExternalInput")
    # w_proj^T * WPS stripes: [tt][p][o]
    wpt_d = nc.dram_tensor("wpt", [TC, P, C], f16, kind="ExternalInput")
    # host-precomputed vs/WPS, replicated across partitions: [p][s]
    vsf_d = nc.dram_tensor("vsf", [bpc, P, S], f16, kind="ExternalInput")
    out_d = nc.dram_tensor("out", [bpc, C, S], f16, kind="ExternalOutput")

    with tile.TileContext(nc) as tc:
        with (
            tc.tile_pool(name="weights", bufs=1) as wpool,
            tc.tile_pool(name="x8", bufs=2) as x8pool,
            tc.tile_pool(name="xpb", bufs=2) as xppool,
            tc.tile_pool(name="y", bufs=1) as ypool,
            tc.tile_pool(name="e", bufs=1) as epool,
            tc.tile_pool(name="wpts", bufs=1) as wptspool,
            tc.tile_pool(name="vsb", bufs=2) as vpool,
            tc.tile_pool(name="osb", bufs=4) as opool,
            tc.tile_pool(name="small", bufs=40) as spool,
            tc.tile_pool(name="psA", bufs=3, space="PSUM") as psA,
            tc.tile_pool(name="psB", bufs=3, space="PSUM") as psB,
            tc.tile_pool(name="psC", bufs=2, space="PSUM") as psC,
        ):
            # warm the PE clock (HAM) with throwaway matmuls on a memset
            # tile — no DMA dependency, so they start immediately
            wz = wpool.tile([P, P], f16, tag="wz")
            nc.vector.memset(wz[:], 0.25)
            ln4t = wpool.tile([P, 1], f32, tag="ln4")
            nc.vector.memset(ln4t[:], -LN4)
            wu = psA.tile([P, NN], f32, tag="psA")
            for _ in range(N_WARMUP):
                nc.tensor.matmul(
                    wu[:, 0:64], wz[:], wz[:, 0:64],
                    start=True, stop=True,
                )
            a16_sb = wpool.tile([P, KC, QC, 2, P], fp8, tag="a16")
            wpt_sb = wpool.tile([P, TC, C], f16, tag="wpt")
            x8_next = xpb_next = vsb_next = None

            for b in range(bpc):
                if b == 0:
                    x8t = x8pool.tile([P, QC, 2, S], fp8, tag="x8")
                    xpb = xppool.tile([P, KC, S], f16, tag="xpb")
                    vsb = vpool.tile([P, S], f16, tag="vsb")
                    # Critical startup set: a16 on the sync queue, x8 on the
                    # ACT hwdge queue — both issue in parallel; x8 split in
                    # n-halves so the first psum group waits on half the
                    # bytes. Non-critical loads are emitted after the first
                    # matmul so they don't dilute ring bandwidth.
                    nc.sync.dma_start(a16_sb[:, 0:1], a16_d[:, 0:1])
                    for q in range(QC):
                        nc.scalar.dma_start(
                            x8t[:, q, :, 0:NN], x8_d[b, q, :, :, 0:NN]
                        )
                    nc.sync.dma_start(a16_sb[:, 1:KC], a16_d[:, 1:KC])
                    for q in range(QC):
                        nc.sync.dma_start(
                            x8t[:, q, :, NN:S], x8_d[b, q, :, :, NN:S]
                        )
                else:
                    # tiles + DMAs were issued during the previous batch
                    # (ahead of its output DMAs in the sync queue)
                    x8t, xpb, vsb = x8_next, xpb_next, vsb_next

                # ---- stage A: y16 = (16 M^T) x via fp8 DoubleRow ----
                y8 = ypool.tile([P, QC, 2, S], fp8, tag="y8")
                for n in range(NCH):
                    for mc in range(KC):
                        ps = psA.tile([P, NN], f32, tag="psA")
                        for q in range(QC):
                            mm = nc.tensor.matmul(
                                ps[:],
                                a16_sb[:, mc, q, :, :],
                                x8t[:, q, :, n * NN : (n + 1) * NN],
                                start=(q == 0),
                                stop=(q == QC - 1),
                                perf_mode=mybir.MatmulPerfMode.DoubleRow,
                            )
                            if b == 0 and mc == 4 and q == 0 and n == 1:
                                # by n=1 mc=4 every critical startup byte
                                # has landed and been consumed for a while
                                h1_mm = mm.ins
                        # y8 copies on ACT: DVE's in-order queue is still
                        # draining the previous batch's stage-C osb chain,
                        # which would stall these (and the PE behind them)
                        nc.scalar.activation(
                            y8[:, mc // 2, mc % 2, n * NN : (n + 1) * NN],
                            ps[:],
                            mybir.ActivationFunctionType.Copy,
                        )
                        if b == 0 and n == 0 and mc == 0:
                            # keep the PE busy (HAM warm) while the remaining
                            # A16 stripes stream in
                            wuf = psA.tile([P, NN], f32, tag="psA")
                            for _ in range(N_FILLER):
                                nc.tensor.matmul(
                                    wuf[:, 0:64], wz[:], wz[:, 0:64],
                                    start=True, stop=True,
                                )
                        if b == 0 and n == 1 and mc == 4:
                            # non-critical input loads start only once ALL
                            # critical x8/a16 bytes have landed (n=1 matmul
                            # consumes the x8 h1 half), so they don't steal
                            # ring bandwidth from the critical stream
                            noncrit = [
                                nc.scalar.dma_start(
                                    wpt_sb[:],
                                    wpt_d.rearrange("t p o -> p t o"),
                                ),
                                nc.scalar.dma_start(
                                    xpb[:],
                                    xpb_d.rearrange("b k p s -> b p k s")[b],
                                ),
                                nc.scalar.dma_start(vsb[:], vsf_d[b]),
                            ]
                            for inst in noncrit:
                                add_dep_helper(
                                    inst.ins, h1_mm, sync=True,
                                    reason="startup: after critical DMAs",
                                )

                # ---- stage B: l16 = x8^T y8 (fp8 DR); exp -> e8, row sums;
                # wps8 = wpt * rcp (DVE; ACT is busy with exp in this
                # window) ----
                e8 = epool.tile([P, QC, 2, S], fp8, tag="e8")
                wps8 = wptspool.tile([P, QC, 2, C], fp8, tag="wps8")
                for tt in range(TC):
                    rsh = []
                    for n in range(NCH):
                        psl = psB.tile([P, NN], f32, tag="psB")
                        for q in range(QC):
                            nc.tensor.matmul(
                                psl[:],
                                x8t[:, q, :, tt * P : (tt + 1) * P],
                                y8[:, q, :, n * NN : (n + 1) * NN],
                                start=(q == 0),
                                stop=(q == QC - 1),
                                perf_mode=mybir.MatmulPerfMode.DoubleRow,
                            )
                        rs = spool.tile([P, 1], f32, tag="rs")
                        nc.scalar.activation(
                            e8[:, tt // 2, tt % 2, n * NN : (n + 1) * NN],
                            psl[:],
                            mybir.ActivationFunctionType.Exp,
                            scale=float(SCALE), bias=ln4t[:], accum_out=rs[:],
                        )
                        rsh.append(rs)
                    rst = spool.tile([P, 1], f32, tag="rst")
                    nc.vector.tensor_tensor(
                        rst[:], rsh[0][:], rsh[1][:], mybir.AluOpType.add
                    )
                    rcp = spool.tile([P, 1], f32, tag="rcp")
                    nc.vector.reciprocal(rcp[:], rst[:])
                    if tt == TC - 1:
                        nc.vector.tensor_scalar(
                            wps8[:, tt // 2, tt % 2, 0:P], wpt_sb[:, tt, 0:P],
                            rcp[:], None,
                            mybir.AluOpType.mult,
                        )
                        nc.vector.tensor_scalar(
                            wps8[:, tt // 2, tt % 2, P:C], wpt_sb[:, tt, P:C],
                            rcp[:], None,
                            mybir.AluOpType.mult,
                        )
                    else:
                        nc.vector.tensor_scalar(
                            wps8[:, tt // 2, tt % 2, :], wpt_sb[:, tt, :],
                            rcp[:], None,
                            mybir.AluOpType.mult,
                        )

                # ---- prefetch next batch's inputs (ahead of this batch's
                # output DMAs in the sync queue) ----
                if b + 1 < bpc:
                    x8_next = x8pool.tile([P, QC, 2, S], fp8, tag="x8")
                    xpb_next = xppool.tile([P, KC, S], f16, tag="xpb")
                    vsb_next = vpool.tile([P, S], f16, tag="vsb")
                    nc.sync.dma_start(
                        x8_next[:], x8_d.rearrange("b q p i s -> b p q i s")[b + 1]
                    )
                    nc.sync.dma_start(
                        xpb_next[:], xpb_d.rearrange("b k p s -> b p k s")[b + 1]
                    )
                    nc.sync.dma_start(vsb_next[:], vsf_d[b + 1])

                # ---- stage C: out = (wps8 @ e8) * vs + (x + b) ----
                cpools = (
                    [(psC, "psC"), (psA, "psA"), (psB, "psB")]
                    if b == bpc - 1
                    else [(psC, "psC")]
                )
                for oc in range(KC):
                    for n in range(NCH):
                        cp, ctag = cpools[(oc * NCH + n) % len(cpools)]
                        pso = cp.tile([P, NN], f32, tag=ctag)
                        for q in range(QC):
                            nc.tensor.matmul(
                                pso[:],
                                wps8[:, q, :, oc * P : (oc + 1) * P],
                                e8[:, q, :, n * NN : (n + 1) * NN],
                                start=(q == 0),
                                stop=(q == QC - 1),
                                perf_mode=mybir.MatmulPerfMode.DoubleRow,
                            )
                        # psum downcast so DVE's multiply runs on
                        # all-16-bit operands; the first two go on DVE
                        # (idle at C start) since ACT is still finishing
                        # the last exps of stage B
                        os16 = opool.tile([P, NN], f16, tag="os16")
                        if oc * NCH + n < 2:
                            nc.vector.tensor_copy(out=os16[:], in_=pso[:])
                        else:
                            nc.scalar.activation(
                                os16[:], pso[:],
                                mybir.ActivationFunctionType.Copy,
                            )
                        osb = opool.tile([P, NN], f16, tag="osb")
                        nc.vector.tensor_tensor(
                            osb[:], os16[:], vsb[:, n * NN : (n + 1) * NN],
                            mybir.AluOpType.mult,
                        )
                        nc.vector.tensor_tensor(
                            osb[:], osb[:], xpb[:, oc, n * NN : (n + 1) * NN],
                            mybir.AluOpType.add,
                        )
                        nc.sync.dma_start(
                            out_d[b, oc * P : (oc + 1) * P, n * NN : (n + 1) * NN],
                            osb[:],
                        )
    nc.compile()
    return nc


def _host_prep(w_qkv, w_proj, b_proj):
    wq = w_qkv[0:C].astype(np.float64)
    wk = w_qkv[C : 2 * C].astype(np.float64)
    wv = w_qkv[2 * C : 3 * C]
    # lhsT for y-matmul: a16[d, c] = 16*M[c, d], M = Wq^T Wk => a16 = 16*Wk^T Wq
    a16 = np.clip(A_SCALE * (wk.T @ wq), -240.0, 240.0).astype(
        ml_dtypes.float8_e4m3
    )
    # SBUF layout [p][mc][q][i][m]: contraction d = q*256 + i*128 + p,
    # output col index c = mc*128 + m
    a16_s = np.ascontiguousarray(
        a16.reshape(QC, 2, P, KC, P).transpose(2, 3, 0, 1, 4)
    )
    wvs = wv.sum(axis=0, dtype=np.float64).astype(np.float32)
    # wpt[tt][p][o] = WPS * w_proj[o, t = tt*128 + p]
    wpt_s = np.ascontiguousarray(
        (w_proj.T * WPS).reshape(TC, P, C).astype(np.float16)
    )
    return a16_s, wpt_s, wvs


_NC_CACHE = {}


def _get_nc(bpc=BPC):
    if bpc not in _NC_CACHE:
        _NC_CACHE[bpc] = build_nc(bpc)
    return _NC_CACHE[bpc]


def kernel(x, w_qkv, w_proj, b_proj, _trace=False):
    x = np.asarray(x, dtype=np.float32)
    a16, wpt, wvs = _host_prep(
        np.asarray(w_qkv, np.float32),
        np.asarray(w_proj, np.float32),
        np.asarray(b_proj, np.float32),
    )
    bp = np.asarray(b_proj, np.float32)
    xr_full = x.reshape(B, C, S)
    # fp8 DR layout [b, q, p, i, s]: c = q*256 + i*128 + p
    x8_full = (
        np.clip(xr_full, -240.0, 240.0)
        .astype(ml_dtypes.float8_e4m3)
        .reshape(B, QC, 2, P, S)
        .transpose(0, 1, 3, 2, 4)
    )
    # residual + bias, f16
    xpb_full = (xr_full + bp[None, :, None]).astype(np.float16).reshape(
        B, KC, P, S
    )
    # vs/WPS as an f16 plane replicated across partitions
    vs_full = (np.einsum("c,bcs->bs", wvs, xr_full) / WPS).astype(np.float16)
    vsf_full = np.broadcast_to(vs_full[:, None, :], (B, P, S))
    in_maps = []
    for c in range(N_CORES):
        sl = slice(c * BPC, (c + 1) * BPC)
        in_maps.append(
            {
                "x8": np.ascontiguousarray(x8_full[sl]),
                "xpb": np.ascontiguousarray(xpb_full[sl]),
                "a16": a16,
                "wpt": wpt,
                "vsf": np.ascontiguousarray(vsf_full[sl]),
            }
        )
    nc = _get_nc(BPC)
    res = run_bass_kernel_spmd(
        nc, in_maps, core_ids=list(range(N_CORES)), trace=_trace
    )
    out = np.concatenate([r["out"] for r in res.results], axis=0)
    out = out.astype(np.float32).reshape(B, C, HH, WW)
    if _trace:
        kernel.last_results = res
    return out
